# revision 1
# baseline (speedup 1.0000x reference)
"""Trainium2 Bass kernel for the CAM-threshold-subtract module.

Computation (per sample b):
    idx    = argmax(logits[b, :])                 # over 1000 classes
    cam    = interm[b, :, :, idx]                 # [7,7] gather
    t      = where(cam > 0.5, cam, 0)
    out[b] = vgg[b] - broadcast(t, [7,7,512])

Sharding: pure data parallel, batch 256 -> 8 cores x 32 samples.

Per-core memory traffic is dominated by vgg (3.2MB read) + out (3.2MB
write); interm is NOT streamed - only 49 floats per sample are fetched
with one indirect-DMA gather (32 descriptors), using a combined index
b*1000 + idx into a [32,1000,49]-strided logical view of interm.
"""

import numpy as np

M = 8          # cores
B = 32         # samples per core
S = 49         # spatial positions (7*7)
C = 512        # vgg channels
K = 1000       # classes
P = 128        # partitions
ROWS = B * S   # 1568 (b,pos) rows per core
NFULL = ROWS // P          # 12 full [128,512] tiles
REM = ROWS - NFULL * P     # 32 rows in the last tile
NT = NFULL + 1             # 13
THRESH = 0.5
FREE_LOADS = 6  # loads issued immediately; the rest wait for the CAM gather


def _build(loop_n=None, do_t=True, do_load=True, do_sub=True, do_store=True,
           sub_dummy=False, t_depth=4, free_loads=6, stagger_on="gather"):
    import contextlib

    import concourse.bacc as bacc
    import concourse.bass as bass
    import concourse.tile as tile
    from concourse import mybir

    nc = bacc.Bacc("TRN2", target_bir_lowering=False, debug=False)
    vgg = nc.dram_tensor("vgg", [ROWS, C], mybir.dt.float32, kind="ExternalInput")
    # interm is pre-transposed on host to [B, K, S] so each CAM row
    # (one channel's 49 spatial values) is contiguous for the row-gather.
    interm = nc.dram_tensor("interm", [B, K, S], mybir.dt.float32, kind="ExternalInput")
    logits = nc.dram_tensor("logits", [B, K], mybir.dt.float32, kind="ExternalInput")
    out = nc.dram_tensor("out", [ROWS, C], mybir.dt.float32, kind="ExternalOutput")

    with tile.TileContext(nc) as tc:
        with (
            tc.tile_pool(name="big", bufs=NT) as big,
            tc.tile_pool(name="small", bufs=1) as small,
            tc.tile_pool(name="dram", bufs=1, space="DRAM") as dpool,
            tc.For_i(0, loop_n) if loop_n else contextlib.nullcontext(),
        ):
            if do_t:
                # The t-path's small DMAs live on the scalar/gpsimd rings so
                # they never queue behind the 13 big vgg loads (sync ring is
                # FIFO per engine).
                # ---- per-sample argmax over class logits ----
                lg = small.tile([B, K], mybir.dt.float32)
                nc.scalar.dma_start(out=lg[:], in_=logits.ap()[:, :])
                mx = small.tile([B, 8], mybir.dt.float32)
                nc.vector.max(mx[:], lg[:])
                mi = small.tile([B, 8], mybir.dt.uint32)
                nc.vector.max_index(mi[:], mx[:], lg[:])

                # combined row index into interm viewed [B*K, S]: b*1000 + idx_b
                base = small.tile([B, 1], mybir.dt.uint32)
                nc.gpsimd.iota(base[:], [[1, 1]], base=0, channel_multiplier=K)
                comb = small.tile([B, 1], mybir.dt.uint32)
                nc.vector.tensor_tensor(
                    out=comb[:], in0=mi[:, 0:1], in1=base[:], op=mybir.AluOpType.add
                )

                gather_inst = None
                if t_depth >= 2:
                    # ---- row-gather the CAM: cam[b,:] = interm[b, idx_b, :] ----
                    # one descriptor per sample, 49 contiguous floats each
                    cam = small.tile([B, S], mybir.dt.float32)
                    gather_inst = nc.gpsimd.indirect_dma_start(
                        out=cam[:],
                        out_offset=None,
                        in_=interm.ap().rearrange("b k s -> (b k) s"),
                        in_offset=bass.IndirectOffsetOnAxis(
                            ap=comb[:, 0:1], axis=0
                        ),
                    )

                if t_depth >= 3:
                    # ---- threshold: t = cam * (cam > 0.5) ----
                    mask = small.tile([B, S], mybir.dt.float32)
                    nc.vector.tensor_scalar(
                        out=mask[:], in0=cam[:], scalar1=THRESH, scalar2=None,
                        op0=mybir.AluOpType.is_gt,
                    )
                    tt = small.tile([B, S], mybir.dt.float32)
                    nc.vector.tensor_tensor(
                        out=tt[:], in0=cam[:], in1=mask[:],
                        op=mybir.AluOpType.mult,
                    )
                    # ---- refold t [32,49] -> [128,13] via DRAM bounce ----
                    # (row g = b*49+pos; tile k holds rows 128k..128k+127)
                    td = dpool.tile([NT, P], mybir.dt.float32)  # flat [1664]
                    bounce_inst = nc.gpsimd.dma_start(
                        out=td[:].flatten()[0:ROWS].rearrange("(b s) -> b s", b=B),
                        in_=tt[:],
                    )

                if t_depth >= 4:
                    # reload split in two so no pad elements are ever touched:
                    # [128,12] strided main block + [32,1] corner of tile 12
                    t_all = small.tile([P, NT], mybir.dt.float32)
                    nc.scalar.dma_start(
                        out=t_all[:, 0:NFULL], in_=td[0:NFULL, :].transpose([1, 0])
                    )
                    nc.gpsimd.dma_start(
                        out=t_all[0:REM, NFULL:NT],
                        in_=td[NFULL:NT, 0:REM].transpose([1, 0]),
                    )
                else:
                    t_all = small.tile([P, NT], mybir.dt.float32)
                    nc.vector.memset(t_all[:], 0.0)
            else:
                gather_inst = None
                t_all = small.tile([P, NT], mybir.dt.float32)
                nc.vector.memset(t_all[:], 0.0)
            if sub_dummy:
                t_all = small.tile([P, NT], mybir.dt.float32, tag="t_dummy")
                nc.vector.memset(t_all[:], 0.0)

            # ---- main stream: out = vgg - t (per-row scalar broadcast) ----
            for k in range(NT if (do_load or do_sub or do_store) else 0):
                rows = P if k < NFULL else REM
                vt = big.tile([P, C], mybir.dt.float32, tag="vt")
                if do_load:
                    ld = nc.sync.dma_start(
                        out=vt[:rows, :], in_=vgg.ap()[k * P : k * P + rows, :]
                    )
                    # hold back later loads so the t-chain's small DMAs
                    # complete at idle-HBM latency instead of queueing
                    # behind 3MB of load descriptors
                    hold = None
                    if stagger_on == "gather":
                        hold = gather_inst
                    elif stagger_on == "bounce" and do_t and t_depth >= 3:
                        hold = bounce_inst
                    if hold is not None and k >= free_loads:
                        bass._add_dep_helper(
                            ld.ins, hold.ins, sync=True,
                            reason="stagger loads behind t-chain",
                        )
                else:
                    nc.vector.memset(vt[:rows, :], 0.0)
                if do_sub:
                    nc.vector.tensor_scalar(
                        out=vt[:rows, :], in0=vt[:rows, :],
                        scalar1=t_all[:rows, k : k + 1], scalar2=None,
                        op0=mybir.AluOpType.subtract,
                    )
                if do_store:
                    nc.scalar.dma_start(
                        out=out.ap()[k * P : k * P + rows, :], in_=vt[:rows, :]
                    )
    nc.compile()
    return nc


def _build_v2(loop_n=None, free_loads=99, sim_safe=False,
              do_t=True, do_load=True, do_sub=True, do_store=True):
    """Bounce-free design.

    16 dense tiles of [98, 512], tile k = samples {2k, 2k+1} (partition
    49*b2 + s).  The CAM fold is done on-chip: PE-transpose [32,49] ->
    PSUM [49,32], threshold, then two strided DVE copies build the
    [98,16] per-partition scalar table.  The t-chain has only two DMA
    links (logits load, CAM gather).
    """
    import contextlib

    import concourse.bacc as bacc
    import concourse.bass as bass
    import concourse.tile as tile
    from concourse import mybir
    from concourse.masks import make_identity

    KT = B // 2      # 16 tiles, 2 samples each
    SP = 64          # sample B's rows sit at partition base 64 (HW-aligned)
    RV = SP + S      # 113 partitions carry data (rows 49..63 are filler)

    nc = bacc.Bacc("TRN2", target_bir_lowering=False, debug=False)
    vgg = nc.dram_tensor("vgg", [ROWS, C], mybir.dt.float32, kind="ExternalInput")
    interm = nc.dram_tensor("interm", [B, K, S], mybir.dt.float32, kind="ExternalInput")
    logits = nc.dram_tensor("logits", [B, K], mybir.dt.float32, kind="ExternalInput")
    out = nc.dram_tensor("out", [ROWS, C], mybir.dt.float32, kind="ExternalOutput")

    with tile.TileContext(nc) as tc:
        with (
            tc.tile_pool(name="big", bufs=KT) as big,
            tc.tile_pool(name="small", bufs=1) as small,
            tc.tile_pool(name="psum", bufs=1, space="PSUM") as psum,
            tc.For_i(0, loop_n) if loop_n else contextlib.nullcontext(),
        ):
            if do_t:
                ident = small.tile([B, B], mybir.dt.float32)
                make_identity(nc, ident[:])

                # ---- per-sample argmax over class logits ----
                lg = small.tile([B, K], mybir.dt.float32)
                nc.scalar.dma_start(out=lg[:], in_=logits.ap()[:, :])
                mx = small.tile([B, 8], mybir.dt.float32)
                nc.vector.max(mx[:], lg[:])
                mi = small.tile([B, 8], mybir.dt.uint32)
                nc.vector.max_index(mi[:], mx[:], lg[:])
                base = small.tile([B, 1], mybir.dt.uint32)
                nc.gpsimd.iota(base[:], [[1, 1]], base=0, channel_multiplier=K)
                comb = small.tile([B, 1], mybir.dt.uint32)
                nc.vector.tensor_tensor(
                    out=comb[:], in0=mi[:, 0:1], in1=base[:],
                    op=mybir.AluOpType.add,
                )

                # ---- row-gather the CAM: cam[b,:] = interm[b, idx_b, :] ----
                cam = small.tile([B, S], mybir.dt.float32)
                gather_inst = nc.gpsimd.indirect_dma_start(
                    out=cam[:],
                    out_offset=None,
                    in_=interm.ap().rearrange("b k s -> (b k) s"),
                    in_offset=bass.IndirectOffsetOnAxis(ap=comb[:, 0:1], axis=0),
                )

                # ---- threshold, then fold on-chip ----
                mask = small.tile([B, S], mybir.dt.float32)
                nc.vector.tensor_scalar(
                    out=mask[:], in0=cam[:], scalar1=THRESH, scalar2=None,
                    op0=mybir.AluOpType.is_gt,
                )
                ttv = small.tile([B, S], mybir.dt.float32)
                nc.vector.tensor_tensor(
                    out=ttv[:], in0=cam[:], in1=mask[:], op=mybir.AluOpType.mult
                )
                # PE transpose: [32, 49] -> PSUM [49, 32]
                pt = psum.tile([S, B], mybir.dt.float32)
                nc.tensor.transpose(pt[:], ttv[:], ident[:])
                # scalar table ct[p, k]: rows 0..48 = t[2k], rows 64..112 =
                # t[2k+1]; filler rows 49..63 zeroed (32-aligned window)
                ct = small.tile([P, KT], mybir.dt.float32)
                nc.vector.memset(ct[32:SP, :], 0.0)
                nc.vector.tensor_copy(out=ct[0:S, :], in_=pt[:, 0:B:2])
                nc.vector.tensor_copy(out=ct[SP:RV, :], in_=pt[:, 1:B:2])
            else:
                gather_inst = None
                ct = small.tile([P, KT], mybir.dt.float32)
                nc.vector.memset(ct[:], 0.0)

            # ---- main stream ----
            vgg3 = vgg.ap().rearrange("(b s) c -> b s c", s=S)
            out3 = out.ap().rearrange("(b s) c -> b s c", s=S)
            for k in range(KT if (do_load or do_sub or do_store) else 0):
                vt = big.tile([P, C], mybir.dt.float32, tag="vt")
                if do_load:
                    for b2 in range(2):
                        ld = nc.sync.dma_start(
                            out=vt[SP * b2 : SP * b2 + S, :],
                            in_=vgg3[2 * k + b2, :, :],
                        )
                        if gather_inst is not None and k >= free_loads:
                            bass._add_dep_helper(
                                ld.ins, gather_inst.ins, sync=True,
                                reason="stagger loads behind CAM gather",
                            )
                else:
                    nc.vector.memset(vt[:], 0.0)
                if not do_sub:
                    pass
                elif sim_safe:
                    # CoreSim refuses reads of uninitialized SBUF, so split
                    # the subtract over the two valid partition windows
                    for b2 in range(2):
                        nc.vector.tensor_scalar(
                            out=vt[SP * b2 : SP * b2 + S, :],
                            in0=vt[SP * b2 : SP * b2 + S, :],
                            scalar1=ct[SP * b2 : SP * b2 + S, k : k + 1],
                            scalar2=None,
                            op0=mybir.AluOpType.subtract,
                        )
                else:
                    # single op over partitions 0..112; filler rows 49..63
                    # hold junk that is computed on but never stored
                    nc.vector.tensor_scalar(
                        out=vt[0:RV, :], in0=vt[0:RV, :],
                        scalar1=ct[0:RV, k : k + 1], scalar2=None,
                        op0=mybir.AluOpType.subtract,
                    )
                if do_store:
                    for b2 in range(2):
                        nc.scalar.dma_start(
                            out=out3[2 * k + b2, :, :],
                            in_=vt[SP * b2 : SP * b2 + S, :],
                        )
    nc.compile()
    return nc


def _build_v3(loop_n=None, free_loads=99,
              do_t=True, do_load=True, do_sub=True, do_store=True):
    """Dense-tile design with an on-chip CAM fold.

    16 dense tiles of [98, 512] (tile k = samples {2k, 2k+1}), one
    contiguous load/store DMA each.  t-chain: logits load -> argmax ->
    row-gather CAM [32,49] -> threshold -> PE transpose to PSUM [49,32]
    -> even half of the scalar table via a strided DVE copy (base 0),
    odd half via a small SBUF->SBUF DMA to partition base 49.
    """
    import contextlib

    import concourse.bacc as bacc
    import concourse.bass as bass
    import concourse.tile as tile
    from concourse import mybir
    from concourse.masks import make_identity

    KT = B // 2      # 16 tiles, 2 samples each
    RT = 2 * S       # 98 rows per tile

    nc = bacc.Bacc("TRN2", target_bir_lowering=False, debug=False)
    vgg = nc.dram_tensor("vgg", [ROWS, C], mybir.dt.float32, kind="ExternalInput")
    interm = nc.dram_tensor("interm", [B, K, S], mybir.dt.float32, kind="ExternalInput")
    logits = nc.dram_tensor("logits", [B, K], mybir.dt.float32, kind="ExternalInput")
    out = nc.dram_tensor("out", [ROWS, C], mybir.dt.float32, kind="ExternalOutput")

    with tile.TileContext(nc) as tc:
        with (
            tc.tile_pool(name="big", bufs=KT) as big,
            tc.tile_pool(name="small", bufs=1) as small,
            tc.tile_pool(name="psum", bufs=1, space="PSUM") as psum,
            tc.For_i(0, loop_n) if loop_n else contextlib.nullcontext(),
        ):
            if do_t:
                ident = small.tile([B, B], mybir.dt.float32)
                make_identity(nc, ident[:])

                lg = small.tile([B, K], mybir.dt.float32)
                nc.scalar.dma_start(out=lg[:], in_=logits.ap()[:, :])
                mx = small.tile([B, 8], mybir.dt.float32)
                nc.vector.max(mx[:], lg[:])
                mi = small.tile([B, 8], mybir.dt.uint32)
                nc.vector.max_index(mi[:], mx[:], lg[:])
                base = small.tile([B, 1], mybir.dt.uint32)
                nc.gpsimd.iota(base[:], [[1, 1]], base=0, channel_multiplier=K)
                comb = small.tile([B, 1], mybir.dt.uint32)
                nc.vector.tensor_tensor(
                    out=comb[:], in0=mi[:, 0:1], in1=base[:],
                    op=mybir.AluOpType.add,
                )

                cam = small.tile([B, S], mybir.dt.float32)
                gather_inst = nc.gpsimd.indirect_dma_start(
                    out=cam[:],
                    out_offset=None,
                    in_=interm.ap().rearrange("b k s -> (b k) s"),
                    in_offset=bass.IndirectOffsetOnAxis(ap=comb[:, 0:1], axis=0),
                )

                mask = small.tile([B, S], mybir.dt.float32)
                nc.vector.tensor_scalar(
                    out=mask[:], in0=cam[:], scalar1=THRESH, scalar2=None,
                    op0=mybir.AluOpType.is_gt,
                )
                ttv = small.tile([B, S], mybir.dt.float32)
                nc.vector.tensor_tensor(
                    out=ttv[:], in0=cam[:], in1=mask[:], op=mybir.AluOpType.mult
                )
                pt = psum.tile([S, B], mybir.dt.float32)
                nc.tensor.transpose(pt[:], ttv[:], ident[:])

                # dense scalar table ct[49*b2+s, k] = t[2k+b2, s]
                ct = small.tile([RT, KT], mybir.dt.float32)
                nc.vector.tensor_copy(out=ct[0:S, :], in_=pt[:, 0:B:2])
                podd = small.tile([S, KT], mybir.dt.float32)
                nc.vector.tensor_copy(out=podd[:], in_=pt[:, 1:B:2])
                nc.gpsimd.dma_start(out=ct[S:RT, :], in_=podd[:])
            else:
                gather_inst = None
                ct = small.tile([RT, KT], mybir.dt.float32)
                nc.vector.memset(ct[:], 0.0)

            # ---- main stream: dense [98, 512] tiles ----
            for k in range(KT if (do_load or do_sub or do_store) else 0):
                vt = big.tile([RT, C], mybir.dt.float32, tag="vt")
                if do_load:
                    ld = nc.sync.dma_start(
                        out=vt[:], in_=vgg.ap()[k * RT : (k + 1) * RT, :]
                    )
                    if gather_inst is not None and k >= free_loads:
                        bass._add_dep_helper(
                            ld.ins, gather_inst.ins, sync=True,
                            reason="stagger loads behind CAM gather",
                        )
                else:
                    nc.vector.memset(vt[:], 0.0)
                if do_sub:
                    nc.vector.tensor_scalar(
                        out=vt[:], in0=vt[:], scalar1=ct[:, k : k + 1],
                        scalar2=None, op0=mybir.AluOpType.subtract,
                    )
                if do_store:
                    nc.scalar.dma_start(
                        out=out.ap()[k * RT : (k + 1) * RT, :], in_=vt[:]
                    )
    nc.compile()
    return nc


def _consts_np():
    """Selection masks for the on-PE CAM fold.

    Column j of the table corresponds to flat row g=j (tile j//128,
    partition j%128).  BSEL[b, j] = 1 iff sample b owns row j;
    SELMASK[s, j] = 1 iff position s matches row j.  Columns j >= 1568
    are zero (tile 12 pad), making the folded values there exactly 0.
    """
    j = np.arange(NT * P)
    valid = j < ROWS
    bsel = (j // S == np.arange(B)[:, None]) & valid
    smask = (j % S == np.arange(S)[:, None]) & valid
    return np.concatenate([bsel, smask], 0).astype(np.float32)  # [81, 1664]


def _build_v6(loop_n=None,
              do_t=True, do_load=True, do_sub=True, do_store=True):
    """Dense stream + on-PE CAM fold via constant selection masks.

    Stream: 13 dense [128,512] tiles (line-rate DMA).  t-chain: logits
    load -> argmax -> row-gather CAM [32,49] -> threshold -> P1 =
    tt^T @ BSEL (4 matmuls) -> mask-mult by SELMASK -> column-sum
    matmuls -> ct [128,13] per-partition scalars.  Only two DMA links
    (logits, gather) on the critical chain; everything partition-aligned.
    """
    import contextlib

    import concourse.bacc as bacc
    import concourse.bass as bass
    import concourse.tile as tile
    from concourse import mybir

    W = NT * P  # 1664 table columns

    nc = bacc.Bacc("TRN2", target_bir_lowering=False, debug=False)
    vgg = nc.dram_tensor("vgg", [ROWS, C], mybir.dt.float32, kind="ExternalInput")
    interm = nc.dram_tensor("interm", [B, K, S], mybir.dt.float32, kind="ExternalInput")
    logits = nc.dram_tensor("logits", [B, K], mybir.dt.float32, kind="ExternalInput")
    consts = nc.dram_tensor("consts", [B + S, W], mybir.dt.float32, kind="ExternalInput")
    out = nc.dram_tensor("out", [ROWS, C], mybir.dt.float32, kind="ExternalOutput")

    with tile.TileContext(nc) as tc:
        with (
            tc.tile_pool(name="big", bufs=NT) as big,
            tc.tile_pool(name="small", bufs=1) as small,
            tc.tile_pool(name="psum", bufs=1, space="PSUM") as psum,
            tc.For_i(0, loop_n) if loop_n else contextlib.nullcontext(),
        ):
            if do_t:
                # selection masks + all-ones vector (off the critical chain)
                bsel = small.tile([B, W], mybir.dt.float32)
                nc.scalar.dma_start(out=bsel[:], in_=consts.ap()[0:B, :])
                smask = small.tile([S, W], mybir.dt.float32)
                nc.scalar.dma_start(out=smask[:], in_=consts.ap()[B : B + S, :])
                ones = small.tile([S, 1], mybir.dt.float32)
                nc.gpsimd.memset(ones[:], 1.0)

                # logits first on the sync ring, ahead of the vgg loads
                lg = small.tile([B, K], mybir.dt.float32)
                nc.sync.dma_start(out=lg[:], in_=logits.ap()[:, :])
                mx = small.tile([B, 8], mybir.dt.float32)
                nc.vector.max(mx[:], lg[:])
                mi = small.tile([B, 8], mybir.dt.uint32)
                nc.vector.max_index(mi[:], mx[:], lg[:])
                base = small.tile([B, 1], mybir.dt.uint32)
                nc.gpsimd.iota(base[:], [[1, 1]], base=0, channel_multiplier=K)
                comb = small.tile([B, 1], mybir.dt.uint32)
                nc.vector.tensor_tensor(
                    out=comb[:], in0=mi[:, 0:1], in1=base[:],
                    op=mybir.AluOpType.add,
                )

                cam = small.tile([B, S], mybir.dt.float32)
                nc.gpsimd.indirect_dma_start(
                    out=cam[:],
                    out_offset=None,
                    in_=interm.ap().rearrange("b k s -> (b k) s"),
                    in_offset=bass.IndirectOffsetOnAxis(ap=comb[:, 0:1], axis=0),
                )

                mask = small.tile([B, S], mybir.dt.float32)
                nc.vector.tensor_scalar(
                    out=mask[:], in0=cam[:], scalar1=THRESH, scalar2=None,
                    op0=mybir.AluOpType.is_gt,
                )
                ttv = small.tile([B, S], mybir.dt.float32)
                nc.vector.tensor_tensor(
                    out=ttv[:], in0=cam[:], in1=mask[:], op=mybir.AluOpType.mult
                )

                # P1[s, j] = t[b(j), s]  (one-hot matmul over samples)
                p1 = psum.tile([S, W], mybir.dt.float32)
                for q in range(0, W, 512):
                    n = min(512, W - q)
                    nc.tensor.matmul(
                        out=p1[:, q : q + n], lhsT=ttv[:],
                        rhs=bsel[:, q : q + n], start=True, stop=True,
                    )
                # keep only s = s(j), then column-sum -> ct[p, T] = t[g]
                l2 = small.tile([S, W], mybir.dt.float32)
                nc.vector.tensor_tensor(
                    out=l2[:], in0=p1[:], in1=smask[:],
                    op=mybir.AluOpType.mult,
                )
                ctp = psum.tile([P, NT], mybir.dt.float32)
                for T in range(NT):
                    nc.tensor.matmul(
                        out=ctp[:, T : T + 1],
                        lhsT=l2[:, T * P : (T + 1) * P],
                        rhs=ones[:], start=True, stop=True,
                    )
                ct = small.tile([P, NT], mybir.dt.float32)
                nc.vector.tensor_copy(out=ct[:], in_=ctp[:])
            else:
                ct = small.tile([P, NT], mybir.dt.float32)
                nc.vector.memset(ct[:], 0.0)

            # ---- main stream: 13 dense [128, 512] tiles ----
            for k in range(NT if (do_load or do_sub or do_store) else 0):
                rows = P if k < NFULL else REM
                vt = big.tile([P, C], mybir.dt.float32, tag="vt")
                if do_load:
                    nc.sync.dma_start(
                        out=vt[:rows, :], in_=vgg.ap()[k * P : k * P + rows, :]
                    )
                else:
                    nc.vector.memset(vt[:rows, :], 0.0)
                if do_sub:
                    nc.vector.tensor_scalar(
                        out=vt[:rows, :], in0=vt[:rows, :],
                        scalar1=ct[:rows, k : k + 1], scalar2=None,
                        op0=mybir.AluOpType.subtract,
                    )
                if do_store:
                    nc.scalar.dma_start(
                        out=out.ap()[k * P : k * P + rows, :], in_=vt[:rows, :]
                    )
    nc.compile()
    return nc


PADROWS = NT * P  # 1664 (host pads vgg with 96 zero rows)


def _build_v8(loop_n=None, free_tiles=6, nchunk=5, bounds=None,
              hold_on="gather", hold_sync=True, late_free=99, late_on="thresh",
              late_sync=True, thresh_pre=True,
              do_t=True, do_load=True, do_sub=True, do_store=True):
    """Interleaved-tile design: one-DMA CAM fold, dense chunked stream.

    Each sample's 49 rows are host-padded to 52 = 4*13.  Partition
    p = 4b+q of one big [128, 13*512] SBUF tile holds rows s = 13q+T of
    sample b (13 rows, contiguous in the padded DRAM image), so chunk
    loads are single 3-dim line-rate DMAs.  The per-row scalar table
    ct[p, T] = t[b, 13q+T] is the element-stream reshape of the gathered
    CAM [32, 52] -> [128, 13]: one SBUF->SBUF DMA with rectangular
    [32, 4, 13] APs on both sides IS the fold - no transpose, no DRAM
    bounce, no matmul.

    t-chain: logits (sync ring, first) -> argmax -> gather -> remap ->
    threshold.  Loads: tiles < free_tiles issue immediately; later chunks
    hold on the comb-index add so the gather's descriptors reach the DMA
    queue right as the free loads drain (bus never idles).
    """
    import contextlib

    import concourse.bacc as bacc
    import concourse.bass as bass
    import concourse.tile as tile
    from concourse import mybir

    SP = 52         # padded rows per sample (= 4*NT)
    W = NT * C      # 6656 columns of the big SBUF tile

    nc = bacc.Bacc("TRN2", target_bir_lowering=False, debug=False)
    vgg = nc.dram_tensor("vgg", [B, SP, C], mybir.dt.float32, kind="ExternalInput")
    interm = nc.dram_tensor("interm", [B, K, S], mybir.dt.float32, kind="ExternalInput")
    logits = nc.dram_tensor("logits", [B, K], mybir.dt.float32, kind="ExternalInput")
    # padded like vgg: stores mirror the load AP exactly (dense 128-partition
    # walk; partition-skipping APs mis-execute on HWDGE). Host strips pad rows.
    out = nc.dram_tensor("out", [B, SP, C], mybir.dt.float32, kind="ExternalOutput")

    # chunk boundaries in tile units
    if bounds is None:
        bounds = [round(i * NT / nchunk) for i in range(nchunk + 1)]
    chunks = [(bounds[i], bounds[i + 1]) for i in range(len(bounds) - 1) if bounds[i] < bounds[i + 1]]

    with tile.TileContext(nc) as tc:
        with (
            tc.tile_pool(name="big", bufs=1) as big,
            tc.tile_pool(name="small", bufs=1) as small,
            tc.tile_pool(name="dram", bufs=1, space="DRAM") as dpool,
            tc.For_i(0, loop_n) if loop_n else contextlib.nullcontext(),
        ):
            hold_inst = None
            late_inst = None
            if do_t:
                # ---- logits first on the sync ring, ahead of the vgg loads
                lg = small.tile([B, K], mybir.dt.float32)
                nc.sync.dma_start(out=lg[:], in_=logits.ap()[:, :])
                mx = small.tile([B, 8], mybir.dt.float32)
                nc.vector.max(mx[:], lg[:])
                mi = small.tile([B, 8], mybir.dt.uint32)
                nc.vector.max_index(mi[:], mx[:], lg[:])
                base = small.tile([B, 1], mybir.dt.uint32)
                nc.gpsimd.iota(base[:], [[1, 1]], base=0, channel_multiplier=K)
                comb = small.tile([B, 1], mybir.dt.uint32)
                comb_inst = nc.vector.tensor_tensor(
                    out=comb[:], in0=mi[:, 0:1], in1=base[:],
                    op=mybir.AluOpType.add,
                )
                hold_inst = comb_inst

                # ---- gather CAM rows: cam[b, 0:49] = interm[b, idx_b, :] ----
                cam = small.tile([B, SP], mybir.dt.float32)
                nc.vector.memset(cam[:], 0.0)
                gather_inst = nc.gpsimd.indirect_dma_start(
                    out=cam[:, 0:S],
                    out_offset=None,
                    in_=interm.ap().rearrange("b k s -> (b k) s"),
                    in_offset=bass.IndirectOffsetOnAxis(ap=comb[:, 0:1], axis=0),
                )

                ct = small.tile([P, NT], mybir.dt.float32)
                if thresh_pre:
                    # threshold on cam (pad cols are 0 -> stay 0)
                    mask = small.tile([B, SP], mybir.dt.float32)
                    nc.vector.tensor_scalar(
                        out=mask[:], in0=cam[:], scalar1=THRESH, scalar2=None,
                        op0=mybir.AluOpType.is_gt,
                    )
                    thresh_inst = nc.vector.tensor_tensor(
                        out=cam[:], in0=cam[:], in1=mask[:],
                        op=mybir.AluOpType.mult,
                    )

                # ---- fold [32,52] -> [128,13] with one SBUF->SBUF DMA ----
                # Element-stream reshape: ct[4b+q, T] = cam[b, 13q+T].
                # Both APs are rectangular [32, 4, 13] with whole-partition
                # steps, so this is a plain legal copy that IS the fold.
                # DRAM bounce (v1-proven DMA classes): flat image of cam is
                # exactly ct's [128,13] layout, so write it flat and reload
                td = dpool.tile([B, SP], mybir.dt.float32)
                nc.gpsimd.dma_start(out=td[:], in_=cam[:])
                remap_inst = nc.scalar.dma_start(
                    out=ct[:],
                    in_=td[:].flatten().rearrange("(p t) -> p t", p=P),
                )

                if not thresh_pre:
                    # threshold in the folded layout: t = ct * (ct > 0.5)
                    mask = small.tile([P, NT], mybir.dt.float32)
                    nc.vector.tensor_scalar(
                        out=mask[:], in0=ct[:], scalar1=THRESH, scalar2=None,
                        op0=mybir.AluOpType.is_gt,
                    )
                    thresh_inst = nc.vector.tensor_tensor(
                        out=ct[:], in0=ct[:], in1=mask[:], op=mybir.AluOpType.mult
                    )
                # release markers for staggered loads
                markers = {
                    "comb": comb_inst,
                    "gather": gather_inst,
                    "remap": remap_inst,
                    "thresh": thresh_inst,
                }
                if "penop" in (hold_on, late_on):
                    # PE is otherwise idle: a PE nop that waits on the gather
                    # completes right at gather-sem and its own completion
                    # signal is prompt, making a clean late-release marker
                    penop = nc.tensor.nop(nofuse=True, hint="gather_done_pe")
                    bass._add_dep_helper(
                        penop.ins, gather_inst.ins, sync=True,
                        reason="PE nop waits for gather data",
                    )
                    markers["penop"] = penop
                if isinstance(hold_on, str) and hold_on.startswith("nop"):
                    # chain of Pool nops after the gather dispatch: completes
                    # ~61ns*k after the gather's descriptors start generating,
                    # a tunable early-release marker (data has NOT landed)
                    k = int(hold_on[3:] or 1)
                    prev = gather_inst
                    for i in range(k):
                        nop_inst = nc.gpsimd.nop(nofuse=True, hint=f"g_nop{i}")
                        bass._add_dep_helper(
                            nop_inst.ins, prev.ins, sync=False,
                            reason="nop chain marks gather dispatch",
                        )
                        prev = nop_inst
                        markers[f"nop{i+1}"] = nop_inst
                    hold_inst = prev
                else:
                    hold_inst = markers[hold_on]
                late_inst = markers.get(late_on) if late_on else None
            else:
                remap_inst = None
                ct = small.tile([P, NT], mybir.dt.float32)
                nc.vector.memset(ct[:], 0.0)

            # ---- main stream: chunked [32, 4, L, C] views ----
            # vgg52[b, 13q+T, c] <-> bt[4b+q, T*C+c]; both DRAM images are
            # exact [128, 6656] matrices, so every DMA is a plain 2-dim
            # dense-partition copy (same AP class as the proven v1 stream)
            vggw = vgg.ap().rearrange("b (q t) c -> (b q) (t c)", q=4)
            outw = out.ap().rearrange("b (q t) c -> (b q) (t c)", q=4)
            bt = big.tile([P, W], mybir.dt.float32)
            # create ALL loads first: with only 8 round-robin DMAHW sem
            # lanes, interleaving load/store creation makes later loads
            # reuse the lane of a not-yet-issued store (false serialization);
            # loads-first gives loads fresh lanes and stores reuse lanes of
            # loads that completed long before.
            last_ld = None
            for (t0, t1) in chunks:
                if do_load:
                    ld = nc.sync.dma_start(
                        out=bt[:, t0 * C : t1 * C], in_=vggw[:, t0 * C : t1 * C]
                    )
                    last_ld = ld
                    if late_inst is not None and t0 >= late_free:
                        bass._add_dep_helper(
                            ld.ins, late_inst.ins, sync=late_sync,
                            reason="stagger tail loads behind the fold",
                        )
                    elif hold_inst is not None and t0 >= free_tiles:
                        bass._add_dep_helper(
                            ld.ins, hold_inst.ins, sync=hold_sync,
                            reason="stagger loads so the gather slots in",
                        )
                else:
                    nc.vector.memset(bt[:, t0 * C : t1 * C], 0.0)
            first_st = None
            for (t0, t1) in chunks:
                if do_sub:
                    for T in range(t0, t1):
                        nc.vector.tensor_scalar(
                            out=bt[:, T * C : (T + 1) * C],
                            in0=bt[:, T * C : (T + 1) * C],
                            scalar1=ct[:, T : T + 1], scalar2=None,
                            op0=mybir.AluOpType.subtract,
                        )
                if do_store:
                    st = nc.scalar.dma_start(
                        out=outw[:, t0 * C : t1 * C], in_=bt[:, t0 * C : t1 * C]
                    )
                    first_st = first_st or st
    nc.compile()
    return nc


_NC = None


def _get_nc():
    global _NC
    if _NC is None:
        _NC = _BUILDER()
    return _NC


def _shard(vgg_end, interm, branchA_end):
    consts = _consts_np()
    in_maps = []
    for i in range(M):
        sl = slice(i * B, (i + 1) * B)
        in_maps.append(
            {
                "vgg": np.ascontiguousarray(vgg_end[sl], dtype=np.float32).reshape(ROWS, C),
                "interm": np.ascontiguousarray(
                    np.asarray(interm[sl], dtype=np.float32).reshape(B, S, K).transpose(0, 2, 1)
                ),
                "logits": np.ascontiguousarray(branchA_end[sl], dtype=np.float32),
                "consts": consts,
            }
        )
    return in_maps


def _shard_v8(vgg_end, interm, branchA_end):
    in_maps = []
    for i in range(M):
        sl = slice(i * B, (i + 1) * B)
        vgg_i = np.asarray(vgg_end[sl], dtype=np.float32).reshape(B, S, C)
        vgg_pad = np.zeros((B, 52, C), np.float32)
        vgg_pad[:, :S] = vgg_i
        in_maps.append(
            {
                "vgg": vgg_pad,
                "interm": np.ascontiguousarray(
                    np.asarray(interm[sl], dtype=np.float32).reshape(B, S, K).transpose(0, 2, 1)
                ),
                "logits": np.ascontiguousarray(branchA_end[sl], dtype=np.float32),
            }
        )
    return in_maps


V8_CFG = dict(free_tiles=6, hold_on="gather", late_free=99, bounds=[0, 3, 6, 8, 10, 13])

# Proven v1 design, schedule retuned in the cost-model sim: holding the
# staggered loads on the DRAM-bounce write (instead of the CAM gather) with
# 8 free tiles keeps the DMA queue ordered the same but restarts the held
# loads ~1.4us earlier (sim 28281 vs 28892 for the shipped default).
V1_CFG = dict(stagger_on="bounce", free_loads=8)


def _BUILDER(loop_n=None):
    return _build(loop_n=loop_n, **V1_CFG)


_SHARDER = _shard


def kernel(vgg_end, interm, branchA_end):
    from concourse.bass_utils import run_bass_kernel_spmd

    nc = _get_nc()
    in_maps = _SHARDER(np.asarray(vgg_end), np.asarray(interm), np.asarray(branchA_end))
    res = run_bass_kernel_spmd(nc, in_maps, core_ids=list(range(M)))
    return np.concatenate(
        [np.asarray(r["out"]).reshape(-1, C)[:ROWS].reshape(B, 7, 7, C)
         for r in res.results],
        axis=0,
    )



# revision 4
# speedup vs baseline: 1.1841x; 1.1841x over previous
"""Trainium2 Bass kernel for the CAM-threshold-subtract module.

Computation (per sample b):
    idx    = argmax(logits[b, :])                 # over 1000 classes
    cam    = interm[b, :, :, idx]                 # [7,7] gather
    t      = where(cam > 0.5, cam, 0)
    out[b] = vgg[b] - broadcast(t, [7,7,512])

Sharding: pure data parallel, batch 256 -> 8 cores x 32 samples.

Per-core memory traffic is dominated by vgg (3.2MB read) + out (3.2MB
write); interm is NOT streamed - only 49 floats per sample are fetched
with one indirect-DMA gather (32 descriptors), using a combined index
b*1000 + idx into a [32,1000,49]-strided logical view of interm.
"""

import numpy as np

M = 8          # cores
B = 32         # samples per core
S = 49         # spatial positions (7*7)
C = 512        # vgg channels
K = 1000       # classes
P = 128        # partitions
ROWS = B * S   # 1568 (b,pos) rows per core
NFULL = ROWS // P          # 12 full [128,512] tiles
REM = ROWS - NFULL * P     # 32 rows in the last tile
NT = NFULL + 1             # 13
THRESH = 0.5
FREE_LOADS = 6  # loads issued immediately; the rest wait for the CAM gather


def _build(loop_n=None, do_t=True, do_load=True, do_sub=True, do_store=True,
           sub_dummy=False, t_depth=4, free_loads=6, stagger_on="gather"):
    import contextlib

    import concourse.bacc as bacc
    import concourse.bass as bass
    import concourse.tile as tile
    from concourse import mybir

    nc = bacc.Bacc("TRN2", target_bir_lowering=False, debug=False)
    vgg = nc.dram_tensor("vgg", [ROWS, C], mybir.dt.float32, kind="ExternalInput")
    # interm is pre-transposed on host to [B, K, S] so each CAM row
    # (one channel's 49 spatial values) is contiguous for the row-gather.
    interm = nc.dram_tensor("interm", [B, K, S], mybir.dt.float32, kind="ExternalInput")
    logits = nc.dram_tensor("logits", [B, K], mybir.dt.float32, kind="ExternalInput")
    out = nc.dram_tensor("out", [ROWS, C], mybir.dt.float32, kind="ExternalOutput")

    with tile.TileContext(nc) as tc:
        with (
            tc.tile_pool(name="big", bufs=NT) as big,
            tc.tile_pool(name="small", bufs=1) as small,
            tc.tile_pool(name="dram", bufs=1, space="DRAM") as dpool,
            tc.For_i(0, loop_n) if loop_n else contextlib.nullcontext(),
        ):
            if do_t:
                # The t-path's small DMAs live on the scalar/gpsimd rings so
                # they never queue behind the 13 big vgg loads (sync ring is
                # FIFO per engine).
                # ---- per-sample argmax over class logits ----
                lg = small.tile([B, K], mybir.dt.float32)
                nc.scalar.dma_start(out=lg[:], in_=logits.ap()[:, :])
                mx = small.tile([B, 8], mybir.dt.float32)
                nc.vector.max(mx[:], lg[:])
                mi = small.tile([B, 8], mybir.dt.uint32)
                nc.vector.max_index(mi[:], mx[:], lg[:])

                # combined row index into interm viewed [B*K, S]: b*1000 + idx_b
                base = small.tile([B, 1], mybir.dt.uint32)
                nc.gpsimd.iota(base[:], [[1, 1]], base=0, channel_multiplier=K)
                comb = small.tile([B, 1], mybir.dt.uint32)
                nc.vector.tensor_tensor(
                    out=comb[:], in0=mi[:, 0:1], in1=base[:], op=mybir.AluOpType.add
                )

                gather_inst = None
                if t_depth >= 2:
                    # ---- row-gather the CAM: cam[b,:] = interm[b, idx_b, :] ----
                    # one descriptor per sample, 49 contiguous floats each
                    cam = small.tile([B, S], mybir.dt.float32)
                    gather_inst = nc.gpsimd.indirect_dma_start(
                        out=cam[:],
                        out_offset=None,
                        in_=interm.ap().rearrange("b k s -> (b k) s"),
                        in_offset=bass.IndirectOffsetOnAxis(
                            ap=comb[:, 0:1], axis=0
                        ),
                    )

                if t_depth >= 3:
                    # ---- threshold: t = cam * (cam > 0.5) ----
                    mask = small.tile([B, S], mybir.dt.float32)
                    nc.vector.tensor_scalar(
                        out=mask[:], in0=cam[:], scalar1=THRESH, scalar2=None,
                        op0=mybir.AluOpType.is_gt,
                    )
                    tt = small.tile([B, S], mybir.dt.float32)
                    nc.vector.tensor_tensor(
                        out=tt[:], in0=cam[:], in1=mask[:],
                        op=mybir.AluOpType.mult,
                    )
                    # ---- refold t [32,49] -> [128,13] via DRAM bounce ----
                    # (row g = b*49+pos; tile k holds rows 128k..128k+127)
                    td = dpool.tile([NT, P], mybir.dt.float32)  # flat [1664]
                    bounce_inst = nc.gpsimd.dma_start(
                        out=td[:].flatten()[0:ROWS].rearrange("(b s) -> b s", b=B),
                        in_=tt[:],
                    )

                if t_depth >= 4:
                    # reload split in two so no pad elements are ever touched:
                    # [128,12] strided main block + [32,1] corner of tile 12
                    t_all = small.tile([P, NT], mybir.dt.float32)
                    nc.scalar.dma_start(
                        out=t_all[:, 0:NFULL], in_=td[0:NFULL, :].transpose([1, 0])
                    )
                    nc.gpsimd.dma_start(
                        out=t_all[0:REM, NFULL:NT],
                        in_=td[NFULL:NT, 0:REM].transpose([1, 0]),
                    )
                else:
                    t_all = small.tile([P, NT], mybir.dt.float32)
                    nc.vector.memset(t_all[:], 0.0)
            else:
                gather_inst = None
                t_all = small.tile([P, NT], mybir.dt.float32)
                nc.vector.memset(t_all[:], 0.0)
            if sub_dummy:
                t_all = small.tile([P, NT], mybir.dt.float32, tag="t_dummy")
                nc.vector.memset(t_all[:], 0.0)

            # ---- main stream: out = vgg - t (per-row scalar broadcast) ----
            for k in range(NT if (do_load or do_sub or do_store) else 0):
                rows = P if k < NFULL else REM
                vt = big.tile([P, C], mybir.dt.float32, tag="vt")
                if do_load:
                    ld = nc.sync.dma_start(
                        out=vt[:rows, :], in_=vgg.ap()[k * P : k * P + rows, :]
                    )
                    # hold back later loads so the t-chain's small DMAs
                    # complete at idle-HBM latency instead of queueing
                    # behind 3MB of load descriptors
                    hold = None
                    if stagger_on == "gather":
                        hold = gather_inst
                    elif stagger_on == "bounce" and do_t and t_depth >= 3:
                        hold = bounce_inst
                    if hold is not None and k >= free_loads:
                        bass._add_dep_helper(
                            ld.ins, hold.ins, sync=True,
                            reason="stagger loads behind t-chain",
                        )
                else:
                    nc.vector.memset(vt[:rows, :], 0.0)
                if do_sub:
                    nc.vector.tensor_scalar(
                        out=vt[:rows, :], in0=vt[:rows, :],
                        scalar1=t_all[:rows, k : k + 1], scalar2=None,
                        op0=mybir.AluOpType.subtract,
                    )
                if do_store:
                    nc.scalar.dma_start(
                        out=out.ap()[k * P : k * P + rows, :], in_=vt[:rows, :]
                    )
    nc.compile()
    return nc


def _build_v2(loop_n=None, free_loads=99, sim_safe=False,
              do_t=True, do_load=True, do_sub=True, do_store=True):
    """Bounce-free design.

    16 dense tiles of [98, 512], tile k = samples {2k, 2k+1} (partition
    49*b2 + s).  The CAM fold is done on-chip: PE-transpose [32,49] ->
    PSUM [49,32], threshold, then two strided DVE copies build the
    [98,16] per-partition scalar table.  The t-chain has only two DMA
    links (logits load, CAM gather).
    """
    import contextlib

    import concourse.bacc as bacc
    import concourse.bass as bass
    import concourse.tile as tile
    from concourse import mybir
    from concourse.masks import make_identity

    KT = B // 2      # 16 tiles, 2 samples each
    SP = 64          # sample B's rows sit at partition base 64 (HW-aligned)
    RV = SP + S      # 113 partitions carry data (rows 49..63 are filler)

    nc = bacc.Bacc("TRN2", target_bir_lowering=False, debug=False)
    vgg = nc.dram_tensor("vgg", [ROWS, C], mybir.dt.float32, kind="ExternalInput")
    interm = nc.dram_tensor("interm", [B, K, S], mybir.dt.float32, kind="ExternalInput")
    logits = nc.dram_tensor("logits", [B, K], mybir.dt.float32, kind="ExternalInput")
    out = nc.dram_tensor("out", [ROWS, C], mybir.dt.float32, kind="ExternalOutput")

    with tile.TileContext(nc) as tc:
        with (
            tc.tile_pool(name="big", bufs=KT) as big,
            tc.tile_pool(name="small", bufs=1) as small,
            tc.tile_pool(name="psum", bufs=1, space="PSUM") as psum,
            tc.For_i(0, loop_n) if loop_n else contextlib.nullcontext(),
        ):
            if do_t:
                ident = small.tile([B, B], mybir.dt.float32)
                make_identity(nc, ident[:])

                # ---- per-sample argmax over class logits ----
                lg = small.tile([B, K], mybir.dt.float32)
                nc.scalar.dma_start(out=lg[:], in_=logits.ap()[:, :])
                mx = small.tile([B, 8], mybir.dt.float32)
                nc.vector.max(mx[:], lg[:])
                mi = small.tile([B, 8], mybir.dt.uint32)
                nc.vector.max_index(mi[:], mx[:], lg[:])
                base = small.tile([B, 1], mybir.dt.uint32)
                nc.gpsimd.iota(base[:], [[1, 1]], base=0, channel_multiplier=K)
                comb = small.tile([B, 1], mybir.dt.uint32)
                nc.vector.tensor_tensor(
                    out=comb[:], in0=mi[:, 0:1], in1=base[:],
                    op=mybir.AluOpType.add,
                )

                # ---- row-gather the CAM: cam[b,:] = interm[b, idx_b, :] ----
                cam = small.tile([B, S], mybir.dt.float32)
                gather_inst = nc.gpsimd.indirect_dma_start(
                    out=cam[:],
                    out_offset=None,
                    in_=interm.ap().rearrange("b k s -> (b k) s"),
                    in_offset=bass.IndirectOffsetOnAxis(ap=comb[:, 0:1], axis=0),
                )

                # ---- threshold, then fold on-chip ----
                mask = small.tile([B, S], mybir.dt.float32)
                nc.vector.tensor_scalar(
                    out=mask[:], in0=cam[:], scalar1=THRESH, scalar2=None,
                    op0=mybir.AluOpType.is_gt,
                )
                ttv = small.tile([B, S], mybir.dt.float32)
                nc.vector.tensor_tensor(
                    out=ttv[:], in0=cam[:], in1=mask[:], op=mybir.AluOpType.mult
                )
                # PE transpose: [32, 49] -> PSUM [49, 32]
                pt = psum.tile([S, B], mybir.dt.float32)
                nc.tensor.transpose(pt[:], ttv[:], ident[:])
                # scalar table ct[p, k]: rows 0..48 = t[2k], rows 64..112 =
                # t[2k+1]; filler rows 49..63 zeroed (32-aligned window)
                ct = small.tile([P, KT], mybir.dt.float32)
                nc.vector.memset(ct[32:SP, :], 0.0)
                nc.vector.tensor_copy(out=ct[0:S, :], in_=pt[:, 0:B:2])
                nc.vector.tensor_copy(out=ct[SP:RV, :], in_=pt[:, 1:B:2])
            else:
                gather_inst = None
                ct = small.tile([P, KT], mybir.dt.float32)
                nc.vector.memset(ct[:], 0.0)

            # ---- main stream ----
            vgg3 = vgg.ap().rearrange("(b s) c -> b s c", s=S)
            out3 = out.ap().rearrange("(b s) c -> b s c", s=S)
            for k in range(KT if (do_load or do_sub or do_store) else 0):
                vt = big.tile([P, C], mybir.dt.float32, tag="vt")
                if do_load:
                    for b2 in range(2):
                        ld = nc.sync.dma_start(
                            out=vt[SP * b2 : SP * b2 + S, :],
                            in_=vgg3[2 * k + b2, :, :],
                        )
                        if gather_inst is not None and k >= free_loads:
                            bass._add_dep_helper(
                                ld.ins, gather_inst.ins, sync=True,
                                reason="stagger loads behind CAM gather",
                            )
                else:
                    nc.vector.memset(vt[:], 0.0)
                if not do_sub:
                    pass
                elif sim_safe:
                    # CoreSim refuses reads of uninitialized SBUF, so split
                    # the subtract over the two valid partition windows
                    for b2 in range(2):
                        nc.vector.tensor_scalar(
                            out=vt[SP * b2 : SP * b2 + S, :],
                            in0=vt[SP * b2 : SP * b2 + S, :],
                            scalar1=ct[SP * b2 : SP * b2 + S, k : k + 1],
                            scalar2=None,
                            op0=mybir.AluOpType.subtract,
                        )
                else:
                    # single op over partitions 0..112; filler rows 49..63
                    # hold junk that is computed on but never stored
                    nc.vector.tensor_scalar(
                        out=vt[0:RV, :], in0=vt[0:RV, :],
                        scalar1=ct[0:RV, k : k + 1], scalar2=None,
                        op0=mybir.AluOpType.subtract,
                    )
                if do_store:
                    for b2 in range(2):
                        nc.scalar.dma_start(
                            out=out3[2 * k + b2, :, :],
                            in_=vt[SP * b2 : SP * b2 + S, :],
                        )
    nc.compile()
    return nc


def _build_v3(loop_n=None, free_loads=99,
              do_t=True, do_load=True, do_sub=True, do_store=True):
    """Dense-tile design with an on-chip CAM fold.

    16 dense tiles of [98, 512] (tile k = samples {2k, 2k+1}), one
    contiguous load/store DMA each.  t-chain: logits load -> argmax ->
    row-gather CAM [32,49] -> threshold -> PE transpose to PSUM [49,32]
    -> even half of the scalar table via a strided DVE copy (base 0),
    odd half via a small SBUF->SBUF DMA to partition base 49.
    """
    import contextlib

    import concourse.bacc as bacc
    import concourse.bass as bass
    import concourse.tile as tile
    from concourse import mybir
    from concourse.masks import make_identity

    KT = B // 2      # 16 tiles, 2 samples each
    RT = 2 * S       # 98 rows per tile

    nc = bacc.Bacc("TRN2", target_bir_lowering=False, debug=False)
    vgg = nc.dram_tensor("vgg", [ROWS, C], mybir.dt.float32, kind="ExternalInput")
    interm = nc.dram_tensor("interm", [B, K, S], mybir.dt.float32, kind="ExternalInput")
    logits = nc.dram_tensor("logits", [B, K], mybir.dt.float32, kind="ExternalInput")
    out = nc.dram_tensor("out", [ROWS, C], mybir.dt.float32, kind="ExternalOutput")

    with tile.TileContext(nc) as tc:
        with (
            tc.tile_pool(name="big", bufs=KT) as big,
            tc.tile_pool(name="small", bufs=1) as small,
            tc.tile_pool(name="psum", bufs=1, space="PSUM") as psum,
            tc.For_i(0, loop_n) if loop_n else contextlib.nullcontext(),
        ):
            if do_t:
                ident = small.tile([B, B], mybir.dt.float32)
                make_identity(nc, ident[:])

                lg = small.tile([B, K], mybir.dt.float32)
                nc.scalar.dma_start(out=lg[:], in_=logits.ap()[:, :])
                mx = small.tile([B, 8], mybir.dt.float32)
                nc.vector.max(mx[:], lg[:])
                mi = small.tile([B, 8], mybir.dt.uint32)
                nc.vector.max_index(mi[:], mx[:], lg[:])
                base = small.tile([B, 1], mybir.dt.uint32)
                nc.gpsimd.iota(base[:], [[1, 1]], base=0, channel_multiplier=K)
                comb = small.tile([B, 1], mybir.dt.uint32)
                nc.vector.tensor_tensor(
                    out=comb[:], in0=mi[:, 0:1], in1=base[:],
                    op=mybir.AluOpType.add,
                )

                cam = small.tile([B, S], mybir.dt.float32)
                gather_inst = nc.gpsimd.indirect_dma_start(
                    out=cam[:],
                    out_offset=None,
                    in_=interm.ap().rearrange("b k s -> (b k) s"),
                    in_offset=bass.IndirectOffsetOnAxis(ap=comb[:, 0:1], axis=0),
                )

                mask = small.tile([B, S], mybir.dt.float32)
                nc.vector.tensor_scalar(
                    out=mask[:], in0=cam[:], scalar1=THRESH, scalar2=None,
                    op0=mybir.AluOpType.is_gt,
                )
                ttv = small.tile([B, S], mybir.dt.float32)
                nc.vector.tensor_tensor(
                    out=ttv[:], in0=cam[:], in1=mask[:], op=mybir.AluOpType.mult
                )
                pt = psum.tile([S, B], mybir.dt.float32)
                nc.tensor.transpose(pt[:], ttv[:], ident[:])

                # dense scalar table ct[49*b2+s, k] = t[2k+b2, s]
                ct = small.tile([RT, KT], mybir.dt.float32)
                nc.vector.tensor_copy(out=ct[0:S, :], in_=pt[:, 0:B:2])
                podd = small.tile([S, KT], mybir.dt.float32)
                nc.vector.tensor_copy(out=podd[:], in_=pt[:, 1:B:2])
                nc.gpsimd.dma_start(out=ct[S:RT, :], in_=podd[:])
            else:
                gather_inst = None
                ct = small.tile([RT, KT], mybir.dt.float32)
                nc.vector.memset(ct[:], 0.0)

            # ---- main stream: dense [98, 512] tiles ----
            for k in range(KT if (do_load or do_sub or do_store) else 0):
                vt = big.tile([RT, C], mybir.dt.float32, tag="vt")
                if do_load:
                    ld = nc.sync.dma_start(
                        out=vt[:], in_=vgg.ap()[k * RT : (k + 1) * RT, :]
                    )
                    if gather_inst is not None and k >= free_loads:
                        bass._add_dep_helper(
                            ld.ins, gather_inst.ins, sync=True,
                            reason="stagger loads behind CAM gather",
                        )
                else:
                    nc.vector.memset(vt[:], 0.0)
                if do_sub:
                    nc.vector.tensor_scalar(
                        out=vt[:], in0=vt[:], scalar1=ct[:, k : k + 1],
                        scalar2=None, op0=mybir.AluOpType.subtract,
                    )
                if do_store:
                    nc.scalar.dma_start(
                        out=out.ap()[k * RT : (k + 1) * RT, :], in_=vt[:]
                    )
    nc.compile()
    return nc


def _consts_np():
    """Selection masks for the on-PE CAM fold.

    Column j of the table corresponds to flat row g=j (tile j//128,
    partition j%128).  BSEL[b, j] = 1 iff sample b owns row j;
    SELMASK[s, j] = 1 iff position s matches row j.  Columns j >= 1568
    are zero (tile 12 pad), making the folded values there exactly 0.
    """
    j = np.arange(NT * P)
    valid = j < ROWS
    bsel = (j // S == np.arange(B)[:, None]) & valid
    smask = (j % S == np.arange(S)[:, None]) & valid
    return np.concatenate([bsel, smask], 0).astype(np.float32)  # [81, 1664]


def _build_v6(loop_n=None,
              do_t=True, do_load=True, do_sub=True, do_store=True):
    """Dense stream + on-PE CAM fold via constant selection masks.

    Stream: 13 dense [128,512] tiles (line-rate DMA).  t-chain: logits
    load -> argmax -> row-gather CAM [32,49] -> threshold -> P1 =
    tt^T @ BSEL (4 matmuls) -> mask-mult by SELMASK -> column-sum
    matmuls -> ct [128,13] per-partition scalars.  Only two DMA links
    (logits, gather) on the critical chain; everything partition-aligned.
    """
    import contextlib

    import concourse.bacc as bacc
    import concourse.bass as bass
    import concourse.tile as tile
    from concourse import mybir

    W = NT * P  # 1664 table columns

    nc = bacc.Bacc("TRN2", target_bir_lowering=False, debug=False)
    vgg = nc.dram_tensor("vgg", [ROWS, C], mybir.dt.float32, kind="ExternalInput")
    interm = nc.dram_tensor("interm", [B, K, S], mybir.dt.float32, kind="ExternalInput")
    logits = nc.dram_tensor("logits", [B, K], mybir.dt.float32, kind="ExternalInput")
    consts = nc.dram_tensor("consts", [B + S, W], mybir.dt.float32, kind="ExternalInput")
    out = nc.dram_tensor("out", [ROWS, C], mybir.dt.float32, kind="ExternalOutput")

    with tile.TileContext(nc) as tc:
        with (
            tc.tile_pool(name="big", bufs=NT) as big,
            tc.tile_pool(name="small", bufs=1) as small,
            tc.tile_pool(name="psum", bufs=1, space="PSUM") as psum,
            tc.For_i(0, loop_n) if loop_n else contextlib.nullcontext(),
        ):
            if do_t:
                # selection masks + all-ones vector (off the critical chain)
                bsel = small.tile([B, W], mybir.dt.float32)
                nc.scalar.dma_start(out=bsel[:], in_=consts.ap()[0:B, :])
                smask = small.tile([S, W], mybir.dt.float32)
                nc.scalar.dma_start(out=smask[:], in_=consts.ap()[B : B + S, :])
                ones = small.tile([S, 1], mybir.dt.float32)
                nc.gpsimd.memset(ones[:], 1.0)

                # logits first on the sync ring, ahead of the vgg loads
                lg = small.tile([B, K], mybir.dt.float32)
                nc.sync.dma_start(out=lg[:], in_=logits.ap()[:, :])
                mx = small.tile([B, 8], mybir.dt.float32)
                nc.vector.max(mx[:], lg[:])
                mi = small.tile([B, 8], mybir.dt.uint32)
                nc.vector.max_index(mi[:], mx[:], lg[:])
                base = small.tile([B, 1], mybir.dt.uint32)
                nc.gpsimd.iota(base[:], [[1, 1]], base=0, channel_multiplier=K)
                comb = small.tile([B, 1], mybir.dt.uint32)
                nc.vector.tensor_tensor(
                    out=comb[:], in0=mi[:, 0:1], in1=base[:],
                    op=mybir.AluOpType.add,
                )

                cam = small.tile([B, S], mybir.dt.float32)
                nc.gpsimd.indirect_dma_start(
                    out=cam[:],
                    out_offset=None,
                    in_=interm.ap().rearrange("b k s -> (b k) s"),
                    in_offset=bass.IndirectOffsetOnAxis(ap=comb[:, 0:1], axis=0),
                )

                mask = small.tile([B, S], mybir.dt.float32)
                nc.vector.tensor_scalar(
                    out=mask[:], in0=cam[:], scalar1=THRESH, scalar2=None,
                    op0=mybir.AluOpType.is_gt,
                )
                ttv = small.tile([B, S], mybir.dt.float32)
                nc.vector.tensor_tensor(
                    out=ttv[:], in0=cam[:], in1=mask[:], op=mybir.AluOpType.mult
                )

                # P1[s, j] = t[b(j), s]  (one-hot matmul over samples)
                p1 = psum.tile([S, W], mybir.dt.float32)
                for q in range(0, W, 512):
                    n = min(512, W - q)
                    nc.tensor.matmul(
                        out=p1[:, q : q + n], lhsT=ttv[:],
                        rhs=bsel[:, q : q + n], start=True, stop=True,
                    )
                # keep only s = s(j), then column-sum -> ct[p, T] = t[g]
                l2 = small.tile([S, W], mybir.dt.float32)
                nc.vector.tensor_tensor(
                    out=l2[:], in0=p1[:], in1=smask[:],
                    op=mybir.AluOpType.mult,
                )
                ctp = psum.tile([P, NT], mybir.dt.float32)
                for T in range(NT):
                    nc.tensor.matmul(
                        out=ctp[:, T : T + 1],
                        lhsT=l2[:, T * P : (T + 1) * P],
                        rhs=ones[:], start=True, stop=True,
                    )
                ct = small.tile([P, NT], mybir.dt.float32)
                nc.vector.tensor_copy(out=ct[:], in_=ctp[:])
            else:
                ct = small.tile([P, NT], mybir.dt.float32)
                nc.vector.memset(ct[:], 0.0)

            # ---- main stream: 13 dense [128, 512] tiles ----
            for k in range(NT if (do_load or do_sub or do_store) else 0):
                rows = P if k < NFULL else REM
                vt = big.tile([P, C], mybir.dt.float32, tag="vt")
                if do_load:
                    nc.sync.dma_start(
                        out=vt[:rows, :], in_=vgg.ap()[k * P : k * P + rows, :]
                    )
                else:
                    nc.vector.memset(vt[:rows, :], 0.0)
                if do_sub:
                    nc.vector.tensor_scalar(
                        out=vt[:rows, :], in0=vt[:rows, :],
                        scalar1=ct[:rows, k : k + 1], scalar2=None,
                        op0=mybir.AluOpType.subtract,
                    )
                if do_store:
                    nc.scalar.dma_start(
                        out=out.ap()[k * P : k * P + rows, :], in_=vt[:rows, :]
                    )
    nc.compile()
    return nc


PADROWS = NT * P  # 1664 (host pads vgg with 96 zero rows)


def _build_v8(loop_n=None, free_tiles=6, nchunk=5, bounds=None,
              hold_on="gather", hold_sync=True, late_free=99, late_on="thresh",
              late_sync=True, thresh_pre=True,
              do_t=True, do_load=True, do_sub=True, do_store=True):
    """Interleaved-tile design: one-DMA CAM fold, dense chunked stream.

    Each sample's 49 rows are host-padded to 52 = 4*13.  Partition
    p = 4b+q of one big [128, 13*512] SBUF tile holds rows s = 13q+T of
    sample b (13 rows, contiguous in the padded DRAM image), so chunk
    loads are single 3-dim line-rate DMAs.  The per-row scalar table
    ct[p, T] = t[b, 13q+T] is the element-stream reshape of the gathered
    CAM [32, 52] -> [128, 13]: one SBUF->SBUF DMA with rectangular
    [32, 4, 13] APs on both sides IS the fold - no transpose, no DRAM
    bounce, no matmul.

    t-chain: logits (sync ring, first) -> argmax -> gather -> remap ->
    threshold.  Loads: tiles < free_tiles issue immediately; later chunks
    hold on the comb-index add so the gather's descriptors reach the DMA
    queue right as the free loads drain (bus never idles).
    """
    import contextlib

    import concourse.bacc as bacc
    import concourse.bass as bass
    import concourse.tile as tile
    from concourse import mybir

    SP = 52         # padded rows per sample (= 4*NT)
    W = NT * C      # 6656 columns of the big SBUF tile

    nc = bacc.Bacc("TRN2", target_bir_lowering=False, debug=False)
    vgg = nc.dram_tensor("vgg", [B, SP, C], mybir.dt.float32, kind="ExternalInput")
    interm = nc.dram_tensor("interm", [B, K, S], mybir.dt.float32, kind="ExternalInput")
    logits = nc.dram_tensor("logits", [B, K], mybir.dt.float32, kind="ExternalInput")
    # padded like vgg: stores mirror the load AP exactly (dense 128-partition
    # walk; partition-skipping APs mis-execute on HWDGE). Host strips pad rows.
    out = nc.dram_tensor("out", [B, SP, C], mybir.dt.float32, kind="ExternalOutput")

    # chunk boundaries in tile units
    if bounds is None:
        bounds = [round(i * NT / nchunk) for i in range(nchunk + 1)]
    chunks = [(bounds[i], bounds[i + 1]) for i in range(len(bounds) - 1) if bounds[i] < bounds[i + 1]]

    with tile.TileContext(nc) as tc:
        with (
            tc.tile_pool(name="big", bufs=1) as big,
            tc.tile_pool(name="small", bufs=1) as small,
            tc.tile_pool(name="dram", bufs=1, space="DRAM") as dpool,
            tc.For_i(0, loop_n) if loop_n else contextlib.nullcontext(),
        ):
            hold_inst = None
            late_inst = None
            if do_t:
                # ---- logits first on the sync ring, ahead of the vgg loads
                lg = small.tile([B, K], mybir.dt.float32)
                nc.sync.dma_start(out=lg[:], in_=logits.ap()[:, :])
                mx = small.tile([B, 8], mybir.dt.float32)
                nc.vector.max(mx[:], lg[:])
                mi = small.tile([B, 8], mybir.dt.uint32)
                nc.vector.max_index(mi[:], mx[:], lg[:])
                base = small.tile([B, 1], mybir.dt.uint32)
                nc.gpsimd.iota(base[:], [[1, 1]], base=0, channel_multiplier=K)
                comb = small.tile([B, 1], mybir.dt.uint32)
                comb_inst = nc.vector.tensor_tensor(
                    out=comb[:], in0=mi[:, 0:1], in1=base[:],
                    op=mybir.AluOpType.add,
                )
                hold_inst = comb_inst

                # ---- gather CAM rows: cam[b, 0:49] = interm[b, idx_b, :] ----
                cam = small.tile([B, SP], mybir.dt.float32)
                nc.vector.memset(cam[:], 0.0)
                gather_inst = nc.gpsimd.indirect_dma_start(
                    out=cam[:, 0:S],
                    out_offset=None,
                    in_=interm.ap().rearrange("b k s -> (b k) s"),
                    in_offset=bass.IndirectOffsetOnAxis(ap=comb[:, 0:1], axis=0),
                )

                ct = small.tile([P, NT], mybir.dt.float32)
                if thresh_pre:
                    # threshold on cam (pad cols are 0 -> stay 0)
                    mask = small.tile([B, SP], mybir.dt.float32)
                    nc.vector.tensor_scalar(
                        out=mask[:], in0=cam[:], scalar1=THRESH, scalar2=None,
                        op0=mybir.AluOpType.is_gt,
                    )
                    thresh_inst = nc.vector.tensor_tensor(
                        out=cam[:], in0=cam[:], in1=mask[:],
                        op=mybir.AluOpType.mult,
                    )

                # ---- fold [32,52] -> [128,13] with one SBUF->SBUF DMA ----
                # Element-stream reshape: ct[4b+q, T] = cam[b, 13q+T].
                # Both APs are rectangular [32, 4, 13] with whole-partition
                # steps, so this is a plain legal copy that IS the fold.
                # DRAM bounce (v1-proven DMA classes): flat image of cam is
                # exactly ct's [128,13] layout, so write it flat and reload
                td = dpool.tile([B, SP], mybir.dt.float32)
                nc.gpsimd.dma_start(out=td[:], in_=cam[:])
                remap_inst = nc.scalar.dma_start(
                    out=ct[:],
                    in_=td[:].flatten().rearrange("(p t) -> p t", p=P),
                )

                if not thresh_pre:
                    # threshold in the folded layout: t = ct * (ct > 0.5)
                    mask = small.tile([P, NT], mybir.dt.float32)
                    nc.vector.tensor_scalar(
                        out=mask[:], in0=ct[:], scalar1=THRESH, scalar2=None,
                        op0=mybir.AluOpType.is_gt,
                    )
                    thresh_inst = nc.vector.tensor_tensor(
                        out=ct[:], in0=ct[:], in1=mask[:], op=mybir.AluOpType.mult
                    )
                # release markers for staggered loads
                markers = {
                    "comb": comb_inst,
                    "gather": gather_inst,
                    "remap": remap_inst,
                    "thresh": thresh_inst,
                }
                if "penop" in (hold_on, late_on):
                    # PE is otherwise idle: a PE nop that waits on the gather
                    # completes right at gather-sem and its own completion
                    # signal is prompt, making a clean late-release marker
                    penop = nc.tensor.nop(nofuse=True, hint="gather_done_pe")
                    bass._add_dep_helper(
                        penop.ins, gather_inst.ins, sync=True,
                        reason="PE nop waits for gather data",
                    )
                    markers["penop"] = penop
                if isinstance(hold_on, str) and hold_on.startswith("nop"):
                    # chain of Pool nops after the gather dispatch: completes
                    # ~61ns*k after the gather's descriptors start generating,
                    # a tunable early-release marker (data has NOT landed)
                    k = int(hold_on[3:] or 1)
                    prev = gather_inst
                    for i in range(k):
                        nop_inst = nc.gpsimd.nop(nofuse=True, hint=f"g_nop{i}")
                        bass._add_dep_helper(
                            nop_inst.ins, prev.ins, sync=False,
                            reason="nop chain marks gather dispatch",
                        )
                        prev = nop_inst
                        markers[f"nop{i+1}"] = nop_inst
                    hold_inst = prev
                else:
                    hold_inst = markers[hold_on]
                late_inst = markers.get(late_on) if late_on else None
            else:
                remap_inst = None
                ct = small.tile([P, NT], mybir.dt.float32)
                nc.vector.memset(ct[:], 0.0)

            # ---- main stream: chunked [32, 4, L, C] views ----
            # vgg52[b, 13q+T, c] <-> bt[4b+q, T*C+c]; both DRAM images are
            # exact [128, 6656] matrices, so every DMA is a plain 2-dim
            # dense-partition copy (same AP class as the proven v1 stream)
            vggw = vgg.ap().rearrange("b (q t) c -> (b q) (t c)", q=4)
            outw = out.ap().rearrange("b (q t) c -> (b q) (t c)", q=4)
            bt = big.tile([P, W], mybir.dt.float32)
            # create ALL loads first: with only 8 round-robin DMAHW sem
            # lanes, interleaving load/store creation makes later loads
            # reuse the lane of a not-yet-issued store (false serialization);
            # loads-first gives loads fresh lanes and stores reuse lanes of
            # loads that completed long before.
            last_ld = None
            for (t0, t1) in chunks:
                if do_load:
                    ld = nc.sync.dma_start(
                        out=bt[:, t0 * C : t1 * C], in_=vggw[:, t0 * C : t1 * C]
                    )
                    last_ld = ld
                    if late_inst is not None and t0 >= late_free:
                        bass._add_dep_helper(
                            ld.ins, late_inst.ins, sync=late_sync,
                            reason="stagger tail loads behind the fold",
                        )
                    elif hold_inst is not None and t0 >= free_tiles:
                        bass._add_dep_helper(
                            ld.ins, hold_inst.ins, sync=hold_sync,
                            reason="stagger loads so the gather slots in",
                        )
                else:
                    nc.vector.memset(bt[:, t0 * C : t1 * C], 0.0)
            first_st = None
            for (t0, t1) in chunks:
                if do_sub:
                    for T in range(t0, t1):
                        nc.vector.tensor_scalar(
                            out=bt[:, T * C : (T + 1) * C],
                            in0=bt[:, T * C : (T + 1) * C],
                            scalar1=ct[:, T : T + 1], scalar2=None,
                            op0=mybir.AluOpType.subtract,
                        )
                if do_store:
                    st = nc.scalar.dma_start(
                        out=outw[:, t0 * C : t1 * C], in_=bt[:, t0 * C : t1 * C]
                    )
                    first_st = first_st or st
    nc.compile()
    return nc


def _build_v10(loop_n=None, do_t=True, do_load=True, do_sub=True, do_store=True,
               lchunks=((0, 5, 128), (5, 10, 128), (10, 13, 96)),
               schunks=((0, 3, 128), (3, 6, 128), (6, 8, 128), (8, 10, 128),
                        (10, 13, 96)),
               logits_ring="scalar"):
    """Dense padded stream + pure-DVE CAM fold (no fold DMA at all).

    Host pads each sample's 49 rows to 52 = 4*13 and lays the stream as
    one [128, 13*512] image with partition p = 32q+b holding rows
    s = 13q+T of sample b.  The fold t[32,52] -> ct[128,13] is then four
    dense DVE copies ct[32q:32q+32, :] = t[:, 13q:13q+13] — partition-
    offset windows, no DRAM bounce, no transpose, no matmul.

    Pad rows are never moved: the tail col-chunk [10,13) is loaded and
    stored on partitions 0..95 only (dense range), so HBM traffic is the
    exact 3.2MB each way.  The t-chain's two DMAs (logits, gather) ride
    rings the big stream never uses (DVE ring + Pool SWDGE), so no
    stagger scheduling is needed.
    """
    import contextlib

    import concourse.bacc as bacc
    import concourse.bass as bass
    import concourse.tile as tile
    from concourse import mybir

    SP = 52          # padded rows per sample (= 4*NT)
    W = NT * C       # 6656 image columns

    nc = bacc.Bacc("TRN2", target_bir_lowering=False, debug=False)
    vgg = nc.dram_tensor("vgg", [P, W], mybir.dt.float32, kind="ExternalInput")
    interm = nc.dram_tensor("interm", [B, K, S], mybir.dt.float32, kind="ExternalInput")
    logits = nc.dram_tensor("logits", [B, K], mybir.dt.float32, kind="ExternalInput")
    out = nc.dram_tensor("out", [P, W], mybir.dt.float32, kind="ExternalOutput")

    with tile.TileContext(nc) as tc:
        with (
            tc.tile_pool(name="big", bufs=1) as big,
            tc.tile_pool(name="small", bufs=1) as small,
            tc.For_i(0, loop_n) if loop_n else contextlib.nullcontext(),
        ):
            bt = big.tile([P, W], mybir.dt.float32)
            if do_load:
                for (t0, t1, rows) in lchunks:
                    nc.sync.dma_start(
                        out=bt[:rows, t0 * C : t1 * C],
                        in_=vgg.ap()[:rows, t0 * C : t1 * C],
                    )

            ct = small.tile([P, NT], mybir.dt.float32)
            if do_t:
                # pad cols of cam must read 0 after the gather
                cam = small.tile([B, SP], mybir.dt.float32)
                nc.gpsimd.memset(cam[:], 0.0)
                base = small.tile([B, 1], mybir.dt.uint32)
                nc.gpsimd.iota(base[:], [[1, 1]], base=0, channel_multiplier=K)

                # logits on the DVE ring: never queues behind the stream
                lg = small.tile([B, K], mybir.dt.float32)
                getattr(nc, logits_ring).dma_start(out=lg[:], in_=logits.ap()[:, :])
                mx = small.tile([B, 8], mybir.dt.float32)
                nc.vector.max(mx[:], lg[:])
                mi = small.tile([B, 8], mybir.dt.uint32)
                nc.vector.max_index(mi[:], mx[:], lg[:])
                comb = small.tile([B, 1], mybir.dt.uint32)
                nc.vector.tensor_tensor(
                    out=comb[:], in0=mi[:, 0:1], in1=base[:],
                    op=mybir.AluOpType.add,
                )

                nc.gpsimd.indirect_dma_start(
                    out=cam[:, 0:S],
                    out_offset=None,
                    in_=interm.ap().rearrange("b k s -> (b k) s"),
                    in_offset=bass.IndirectOffsetOnAxis(ap=comb[:, 0:1], axis=0),
                )

                # threshold in place: cam = cam * (cam > 0.5)
                mask = small.tile([B, SP], mybir.dt.float32)
                nc.vector.tensor_scalar(
                    out=mask[:], in0=cam[:], scalar1=THRESH, scalar2=None,
                    op0=mybir.AluOpType.is_gt,
                )
                nc.vector.tensor_tensor(
                    out=cam[:], in0=cam[:], in1=mask[:], op=mybir.AluOpType.mult,
                )

                # ---- the fold: four dense DVE copies ----
                for q in range(4):
                    nc.vector.tensor_copy(
                        out=ct[32 * q : 32 * (q + 1), :],
                        in_=cam[:, 13 * q : 13 * (q + 1)],
                    )
            else:
                nc.vector.memset(ct[:], 0.0)

            if do_sub:
                for T in range(NT):
                    rows = P if T < 10 else 96
                    nc.vector.tensor_scalar(
                        out=bt[:rows, T * C : (T + 1) * C],
                        in0=bt[:rows, T * C : (T + 1) * C],
                        scalar1=ct[:rows, T : T + 1], scalar2=None,
                        op0=mybir.AluOpType.subtract,
                    )
            if do_store:
                for (t0, t1, rows) in schunks:
                    nc.scalar.dma_start(
                        out=out.ap()[:rows, t0 * C : t1 * C],
                        in_=bt[:rows, t0 * C : t1 * C],
                    )
    nc.compile()
    return nc


def _shard_v10(vgg_end, interm, branchA_end):
    in_maps = []
    for i in range(M):
        sl = slice(i * B, (i + 1) * B)
        v = np.asarray(vgg_end[sl], dtype=np.float32).reshape(B, S, C)
        img = np.empty((4, B, NT, C), np.float32)  # [q, b, T, c] -> p = 32q+b
        for q in range(4):
            ns = min(NT, S - NT * q)
            img[q, :, :ns] = v[:, NT * q : NT * q + ns]
        in_maps.append(
            {
                "vgg": img.reshape(P, NT * C),
                "interm": np.ascontiguousarray(
                    np.asarray(interm[sl], dtype=np.float32).reshape(B, S, K).transpose(0, 2, 1)
                ),
                "logits": np.ascontiguousarray(branchA_end[sl], dtype=np.float32),
            }
        )
    return in_maps


def _unshard_v10(res):
    outs = []
    for r in res.results:
        o = np.asarray(r["out"]).reshape(4, B, NT, C)
        full = np.empty((B, S, C), np.float32)
        for q in range(4):
            ns = min(NT, S - NT * q)
            full[:, NT * q : NT * q + ns] = o[q, :, :ns]
        outs.append(full.reshape(B, 7, 7, C))
    return np.concatenate(outs, axis=0)


_NC = None


def _get_nc():
    global _NC
    if _NC is None:
        _NC = _BUILDER()
    return _NC


def _shard(vgg_end, interm, branchA_end):
    consts = _consts_np()
    in_maps = []
    for i in range(M):
        sl = slice(i * B, (i + 1) * B)
        in_maps.append(
            {
                "vgg": np.ascontiguousarray(vgg_end[sl], dtype=np.float32).reshape(ROWS, C),
                "interm": np.ascontiguousarray(
                    np.asarray(interm[sl], dtype=np.float32).reshape(B, S, K).transpose(0, 2, 1)
                ),
                "logits": np.ascontiguousarray(branchA_end[sl], dtype=np.float32),
                "consts": consts,
            }
        )
    return in_maps


def _shard_v8(vgg_end, interm, branchA_end):
    in_maps = []
    for i in range(M):
        sl = slice(i * B, (i + 1) * B)
        vgg_i = np.asarray(vgg_end[sl], dtype=np.float32).reshape(B, S, C)
        vgg_pad = np.zeros((B, 52, C), np.float32)
        vgg_pad[:, :S] = vgg_i
        in_maps.append(
            {
                "vgg": vgg_pad,
                "interm": np.ascontiguousarray(
                    np.asarray(interm[sl], dtype=np.float32).reshape(B, S, K).transpose(0, 2, 1)
                ),
                "logits": np.ascontiguousarray(branchA_end[sl], dtype=np.float32),
            }
        )
    return in_maps


V8_CFG = dict(free_tiles=6, hold_on="gather", late_free=99, bounds=[0, 3, 6, 8, 10, 13])

# Proven v1 design, schedule retuned in the cost-model sim: holding the
# staggered loads on the DRAM-bounce write (instead of the CAM gather) with
# 8 free tiles keeps the DMA queue ordered the same but restarts the held
# loads ~1.4us earlier (sim 28281 vs 28892 for the shipped default).
V1_CFG = dict(stagger_on="bounce", free_loads=8)


def _BUILDER(loop_n=None):
    return _build_v10(loop_n=loop_n)


_SHARDER = _shard_v10
_UNSHARDER = _unshard_v10


def kernel(vgg_end, interm, branchA_end):
    from concourse.bass_utils import run_bass_kernel_spmd

    nc = _get_nc()
    in_maps = _SHARDER(np.asarray(vgg_end), np.asarray(interm), np.asarray(branchA_end))
    res = run_bass_kernel_spmd(nc, in_maps, core_ids=list(range(M)))
    return _UNSHARDER(res)

